# revision 1
# baseline (speedup 1.0000x reference)
"""Trainium2 Bass kernel for nn_MoEsparseRoutingForClassification.

Reference computation (B=64, S=128, H=1024, E=8, L=2):
    x = X[:, 0, :]                                   # CLS token [B,H]
    y[b,o]   = sum_e g[b,e] * (x[b] . dense_w[e,o,:]) + (g @ dense_b)[b,o]
    t        = tanh(y)
    out[b,l] = sum_e g[b,e] * (t[b] . out_w[e,l,:])  + (g @ out_b)[b,l]

Distribution: the H output dim of the dense layer is sharded 8 ways
(OC=128 per core).  Core c computes y[:, c*OC:(c+1)*OC] (which needs the
full CLS token but only a 4 MiB slice dense_w[:, c_slice, :]), applies
tanh, and contracts its slice against out_w[:, :, c_slice] to produce a
partial [B,L] logit.  The partials (incl. the out_b bias, fed only to
core 0) sum to the full output on the host.  Total HBM traffic per core
is ~4.3 MiB - the memory roofline for this problem - and no cross-core
collective is needed.

On-core layout: OC=128 is further split into two 64-wide halves mapped
to PSUM partition halves (rows 64h+b).  The two fp32 matmul streams per
k-tile then write disjoint PE column groups, which the PE runs
CONCURRENTLY (fp32 streams at half rate per pass; two col-group streams
recover the full 2 cols/cycle) - stage-1 PE time halves to ~7us, under
the ~13us DMA floor.

Everything arithmetic runs on device; the host only slices, transposes
(layout prep), and sums the partial outputs.
"""

import sys

import numpy as np

for _p in ("/opt/trn_rl_repo",):
    if _p not in sys.path:
        sys.path.insert(0, _p)

# If the environment sets BASS_TRACE but lacks antenv.axon_hooks (this agent
# image does), run_bass_kernel_spmd would crash on import; pre-seed a no-op
# module so tracing degrades gracefully instead.
try:  # pragma: no cover
    import antenv.axon_hooks  # noqa: F401
except Exception:  # pragma: no cover
    import types as _types

    _m = _types.ModuleType("antenv.axon_hooks")
    _m._hook = None
    _m.set_axon_ntff_profile_hook = lambda h: setattr(_m, "_hook", h)
    _m.get_axon_ntff_profile_hook = lambda: _m._hook
    sys.modules["antenv.axon_hooks"] = _m

B, S, H = 64, 128, 1024
E, L = 8, 2
NCORES = 8
OC = H // NCORES          # dense-output slice per core (128)
HC = OC // 2              # half-slice mapped to a PSUM partition half (64)
KT = H // 128             # contraction tiles
P = 128

_cached = None


def _build():
    from contextlib import ExitStack

    import concourse.tile as tile
    from concourse import bacc, mybir

    F32 = mybir.dt.float32
    AF = mybir.ActivationFunctionType
    OP = mybir.AluOpType

    nc = bacc.Bacc("TRN2", target_bir_lowering=False, debug=False,
                   num_devices=NCORES)

    # E-pack along the free dim (one DMA for all E-partition consts):
    #   gt [E,B] | db [E,2,HC] | ow2 [E,2,L,HC] | ob [E,L] | gtz [E,P]
    EPACK = B + OC + L * OC + L + P      # 64+128+256+2+128 = 578
    xt_d = nc.dram_tensor("xt", [P, KT, B], F32, kind="ExternalInput")
    w1_d = nc.dram_tensor("w1", [P, KT, 2, E, HC], F32, kind="ExternalInput")
    ep_d = nc.dram_tensor("ep", [E, EPACK], F32, kind="ExternalInput")
    gc_d = nc.dram_tensor("gc", [P, E], F32, kind="ExternalInput")
    out_d = nc.dram_tensor("out", [P, L], F32, kind="ExternalOutput")

    with tile.TileContext(nc) as tc, ExitStack() as ctx:
        consts = ctx.enter_context(tc.tile_pool(name="consts", bufs=1))
        wpool = ctx.enter_context(tc.tile_pool(name="wpool", bufs=1))
        mixp = ctx.enter_context(tc.tile_pool(name="mixp", bufs=2))
        smallp = ctx.enter_context(tc.tile_pool(name="smallp", bufs=1))
        psy = ctx.enter_context(tc.tile_pool(name="psy", bufs=1, space="PSUM"))
        pss = ctx.enter_context(tc.tile_pool(name="pss", bufs=1, space="PSUM"))

        # The sync ring carries xt then the w1 train (1,1,1,.5,.5 MiB):
        # sequential so the PE chases chunk completions, with a small LAST
        # chunk so the post-DMA matmul trail is a single k-pair.  ep/gc ride
        # the scalar ring concurrently (they only compete with chunk 0 for
        # ~1us and their consumers are idle until then anyway).
        # 4 x 1 MiB chunks measured fastest: more chunks slow the SDMA
        # train (~0.6us per extra boundary) and delay HAM warm-up; fewer
        # chunks starve the PE of overlap.
        xt_t = consts.tile([P, KT, B], F32)
        nc.sync.dma_start(out=xt_t, in_=xt_d.ap())
        w1_t = wpool.tile([P, KT, 2, E, HC], F32)
        for klo, khi in ((0, 2), (2, 4), (4, 6), (6, 8)):
            nc.sync.dma_start(
                out=w1_t[:, klo:khi],
                in_=w1_d.ap()[:, klo:khi],
            )
        ep_t = consts.tile([E, EPACK], F32)
        nc.scalar.dma_start(out=ep_t, in_=ep_d.ap())
        gc_t = consts.tile([P, E], F32)
        nc.scalar.dma_start(out=gc_t, in_=gc_d.ap())
        o = 0
        gt_t = ep_t[:, o:o + B]; o += B
        db_t = ep_t[:, o:o + OC].rearrange("e (h c) -> e h c", h=2); o += OC
        ow_t = ep_t[:, o:o + L * OC].rearrange(
            "e (h l c) -> e h l c", h=2, l=L); o += L * OC
        ob_t = ep_t[:, o:o + L]; o += L
        gtz_t = ep_t[:, o:o + P]                 # gates.T | zeros

        # ---- small matmuls first so their consumers unblock early ----
        # sel_db^h [64h+b, hc] ; sel_ow^h [64h+b, (l, hc)] ; sel_ob [p, l]
        psum_db = pss.tile([P, HC], F32)
        psum_ow = pss.tile([P, L, HC], F32)
        for h in range(2):
            sl = slice(h * 64, h * 64 + 64)
            nc.tensor.matmul(psum_db[sl, :], gt_t, db_t[:, h, :],
                             start=True, stop=True, skip_group_check=True)
            nc.tensor.matmul(
                psum_ow[sl, :, :].rearrange("b l c -> b (l c)"),
                gt_t, ow_t[:, h].rearrange("e l c -> e (l c)"),
                start=True, stop=True, skip_group_check=True,
            )
        psum_ob = pss.tile([P, L], F32)
        nc.tensor.matmul(psum_ob[:], gtz_t, ob_t, start=True, stop=True)
        sdb_t = smallp.tile([P, HC], F32)
        nc.scalar.copy(sdb_t[:], psum_db[:])
        sob_t = smallp.tile([P, L], F32)
        nc.scalar.copy(sob_t[:], psum_ob[:])

        # ---- stage 1: y[64h+b, (e, hc)] = x . dense_w[e, oc_half, :] ----
        # The h=0 / h=1 matmuls write PSUM partition halves 0-63 / 64-127,
        # i.e. disjoint PE col-groups -> the PE overlaps the two fp32
        # streams (2x throughput).  k-outer so the PE consumes each w1
        # chunk as it lands.
        # Gate-broadcast table gb[p, (e, hc)] = g[b, e], built early on the
        # DVE (hidden under the w1 DMA stream).
        ones_t = smallp.tile([P, HC], F32)
        nc.vector.memset(ones_t[:], 1.0)
        gb_t = consts.tile([P, E, HC], F32)
        for e in range(E):
            nc.vector.tensor_scalar_mul(gb_t[:, e, :], ones_t[:],
                                        gc_t[:, e:e + 1])

        psum_y = psy.tile([P, E, HC], F32)
        for k in range(KT):
            for h in range(2):
                nc.tensor.matmul(
                    psum_y[h * 64:h * 64 + 64, :, :].rearrange(
                        "b e c -> b (e c)"),
                    xt_t[:, k, :],
                    w1_t[:, k, h].rearrange("p e c -> p (e c)"),
                    start=(k == 0),
                    stop=(k == KT - 1),
                    skip_group_check=True,
                )

        prod_t = mixp.tile([P, E, HC], F32)
        nc.vector.tensor_tensor(
            out=prod_t[:], in0=psum_y[:], in1=gb_t[:], op=OP.mult,
        )
        # contiguous pairwise tree over e (strided reduce is ~2x slower)
        t1 = mixp.tile([P, 4, HC], F32)
        nc.vector.tensor_add(t1[:], prod_t[:, 0:4, :], prod_t[:, 4:8, :])
        t2 = mixp.tile([P, 2, HC], F32)
        nc.vector.tensor_add(t2[:], t1[:, 0:2, :], t1[:, 2:4, :])
        t3 = mixp.tile([P, HC], F32)
        nc.vector.tensor_add(t3[:], t2[:, 0, :], t2[:, 1, :])
        acc = mixp.tile([P, HC], F32)
        nc.vector.tensor_add(acc[:], t3[:], sdb_t[:])

        t_t = smallp.tile([P, HC], F32)
        nc.scalar.activation(t_t[:], acc[:], AF.Tanh)

        # ---- stage 2: partial[64h+b, l] = sum_hc t * sel_ow (+ sel_ob) ----
        # NOTE: InstTensorTensorReduce faults TRN2; scalar_tensor_tensor with
        # accum_out (free-dim sum) is the reliable path.
        out_t = smallp.tile([P, L], F32)
        pre_t = smallp.tile([P, L], F32)
        dump = smallp.tile([P, HC], F32)
        for l in range(L):
            nc.vector.scalar_tensor_tensor(
                out=dump[:],
                in0=psum_ow[:, l, :],
                scalar=1.0,
                in1=t_t[:],
                op0=OP.mult,
                op1=OP.mult,
                accum_out=pre_t[:, l:l + 1],
            )
        nc.vector.tensor_add(out_t[:], pre_t[:], sob_t[:])

        nc.sync.dma_start(out=out_d.ap(), in_=out_t[:])

    nc.compile()
    return nc


def _prep_inputs(X, gates, dense_w, dense_b, out_w, out_b):
    """Host-side layout prep (slice/transpose only) -> per-core input maps."""
    X = np.asarray(X, dtype=np.float32)
    gates = np.asarray(gates, dtype=np.float32)
    dense_w = np.asarray(dense_w, dtype=np.float32)
    dense_b = np.asarray(dense_b, dtype=np.float32)
    out_w = np.asarray(out_w, dtype=np.float32)
    out_b = np.asarray(out_b, dtype=np.float32)

    xcls = X[:, 0, :]                                     # [B, H]
    # xt[i_lo, k, b] = x[b, k*128 + i_lo]
    xt = np.ascontiguousarray(xcls.T.reshape(KT, P, B).transpose(1, 0, 2))
    gt = np.ascontiguousarray(gates.T)                    # [E, B]
    gtz = np.concatenate([gt, np.zeros_like(gt)], axis=1)  # [E, 128]
    gc2 = np.ascontiguousarray(np.vstack([gates, gates]))  # [128, E]

    in_maps = []
    for c in range(NCORES):
        sl = slice(c * OC, (c + 1) * OC)
        # w1[i_lo, k, h, e, hc] = dense_w[e, c*OC + h*64 + hc, k*128 + i_lo]
        w1 = np.ascontiguousarray(
            dense_w[:, sl, :]                   # [E, OC, H]
            .reshape(E, 2, HC, KT, P)           # [e, h, hc, k, i_lo]
            .transpose(4, 3, 1, 0, 2)           # [i_lo, k, h, e, hc]
        )

        # ow2[e, (h, l, hc)] = out_w[e, l, c*OC + h*64 + hc]
        ow2 = (out_w[:, :, sl].reshape(E, L, 2, HC)
               .transpose(0, 2, 1, 3).reshape(E, L * OC))
        ob = out_b if c == 0 else np.zeros_like(out_b)
        ep = np.ascontiguousarray(
            np.concatenate([gt, dense_b[:, sl], ow2, ob, gtz], axis=1)
        )
        in_maps.append({
            "xt": xt,
            "w1": w1,
            "ep": ep,
            "gc": gc2,
        })
    return in_maps


def _run(in_maps, trace=False, tmpdir=None):
    global _cached
    from concourse.bass_utils import run_bass_kernel_spmd

    if _cached is None:
        _cached = _build()
    res = run_bass_kernel_spmd(
        _cached, in_maps, list(range(NCORES)), trace=trace, tmpdir=tmpdir,
    )
    return res


def kernel(X, gates, dense_w, dense_b, out_w, out_b):
    in_maps = _prep_inputs(X, gates, dense_w, dense_b, out_w, out_b)
    res = _run(in_maps)
    acc = np.zeros((B, L), dtype=np.float64)
    for c in range(NCORES):
        part = res.results[c]["out"].astype(np.float64)   # [128, L]
        acc += part.reshape(2, B, L).sum(axis=0)
    return acc.astype(np.float32)



# revision 6
# speedup vs baseline: 1.2442x; 1.2442x over previous
"""Trainium2 Bass kernel for nn_MoEsparseRoutingForClassification.

Reference computation (B=64, S=128, H=1024, E=8, L=2):
    x = X[:, 0, :]                                   # CLS token [B,H]
    y[b,o]   = sum_e g[b,e] * (x[b] . dense_w[e,o,:]) + (g @ dense_b)[b,o]
    t        = tanh(y)
    out[b,l] = sum_e g[b,e] * (t[b] . out_w[e,l,:])  + (g @ out_b)[b,l]

Distribution: the H output dim of the dense layer is sharded 8 ways
(OC=128 per core).  Core c computes y[:, c*OC:(c+1)*OC] (which needs the
full CLS token but only a 4 MiB slice dense_w[:, c_slice, :]), applies
tanh, and contracts its slice against out_w[:, :, c_slice] to produce a
partial [B,L] logit.  The partials (incl. the out_b bias, fed only to
core 0) sum to the full output on the host.  Total HBM traffic per core
is ~4.3 MiB - the memory roofline for this problem - and no cross-core
collective is needed.

On-core layout: OC=128 is further split into two 64-wide halves mapped
to PSUM partition halves (rows 64h+b).  The two fp32 matmul streams per
k-tile then write disjoint PE column groups, which the PE runs
CONCURRENTLY (fp32 streams at half rate per pass; two col-group streams
recover the full 2 cols/cycle) - stage-1 PE time halves to ~7us, under
the ~13us DMA floor.

Everything arithmetic runs on device; the host only slices, transposes
(layout prep), and sums the partial outputs.
"""

import sys

import numpy as np

for _p in ("/opt/trn_rl_repo",):
    if _p not in sys.path:
        sys.path.insert(0, _p)

# If the environment sets BASS_TRACE but lacks antenv.axon_hooks (this agent
# image does), run_bass_kernel_spmd would crash on import; pre-seed a no-op
# module so tracing degrades gracefully instead.
try:  # pragma: no cover
    import antenv.axon_hooks  # noqa: F401
except Exception:  # pragma: no cover
    import types as _types

    _m = _types.ModuleType("antenv.axon_hooks")
    _m._hook = None
    _m.set_axon_ntff_profile_hook = lambda h: setattr(_m, "_hook", h)
    _m.get_axon_ntff_profile_hook = lambda: _m._hook
    sys.modules["antenv.axon_hooks"] = _m

B, S, H = 64, 128, 1024
E, L = 8, 2
NCORES = 8
OC = H // NCORES          # dense-output slice per core (128)
HC = OC // 2              # half-slice mapped to a PSUM partition half (64)
KT = H // 128             # contraction tiles
P = 128
# w1/xt ship as bf16 (host-side cast): halves the dominant HBM train and
# makes the stage-1 matmuls single-pass.  Measured end-to-end rel err vs
# the fp32 reference: 2.3e-3 (tolerance 2e-2).
W1_CHUNKS = ((0, 4), (4, 7), (7, 8))   # small last chunk -> short PE trail

_cached = None


def _build():
    from contextlib import ExitStack

    import concourse.tile as tile
    from concourse import bacc, mybir

    F32 = mybir.dt.float32
    BF16 = mybir.dt.bfloat16
    AF = mybir.ActivationFunctionType
    OP = mybir.AluOpType

    nc = bacc.Bacc("TRN2", target_bir_lowering=False, debug=False,
                   num_devices=NCORES)

    # E-pack along the free dim (one DMA for all E-partition consts):
    #   gt [E,B] | db [E,2,HC] | ow2 [E,2,L,HC] | ob [E,L] | gtz [E,P]
    EPACK = B + OC + L * OC + L + P      # 64+128+256+2+128 = 578
    xt_d = nc.dram_tensor("xt", [P, KT, B], BF16, kind="ExternalInput")
    w1_d = nc.dram_tensor("w1", [P, KT, 2, E, HC], BF16, kind="ExternalInput")
    ep_d = nc.dram_tensor("ep", [E, EPACK], F32, kind="ExternalInput")
    gc_d = nc.dram_tensor("gc", [P, E], F32, kind="ExternalInput")
    out_d = nc.dram_tensor("out", [P, L], F32, kind="ExternalOutput")

    with tile.TileContext(nc) as tc, ExitStack() as ctx:
        consts = ctx.enter_context(tc.tile_pool(name="consts", bufs=1))
        wpool = ctx.enter_context(tc.tile_pool(name="wpool", bufs=1))
        mixp = ctx.enter_context(tc.tile_pool(name="mixp", bufs=2))
        smallp = ctx.enter_context(tc.tile_pool(name="smallp", bufs=1))
        psy = ctx.enter_context(tc.tile_pool(name="psy", bufs=1, space="PSUM"))
        pss = ctx.enter_context(tc.tile_pool(name="pss", bufs=1, space="PSUM"))

        # The sync ring carries xt then the w1 train (1,1,1,.5,.5 MiB):
        # sequential so the PE chases chunk completions, with a small LAST
        # chunk so the post-DMA matmul trail is a single k-pair.  ep/gc ride
        # the scalar ring concurrently (they only compete with chunk 0 for
        # ~1us and their consumers are idle until then anyway).
        # 4 x 1 MiB chunks measured fastest: more chunks slow the SDMA
        # train (~0.6us per extra boundary) and delay HAM warm-up; fewer
        # chunks starve the PE of overlap.
        xt_t = consts.tile([P, KT, B], BF16)
        nc.sync.dma_start(out=xt_t, in_=xt_d.ap())
        w1_t = wpool.tile([P, KT, 2, E, HC], BF16)
        for klo, khi in W1_CHUNKS:
            nc.sync.dma_start(
                out=w1_t[:, klo:khi],
                in_=w1_d.ap()[:, klo:khi],
            )
        ep_t = consts.tile([E, EPACK], F32)
        nc.scalar.dma_start(out=ep_t, in_=ep_d.ap())
        gc_t = consts.tile([P, E], F32)
        nc.scalar.dma_start(out=gc_t, in_=gc_d.ap())
        o = 0
        gt_t = ep_t[:, o:o + B]; o += B
        db_t = ep_t[:, o:o + OC].rearrange("e (h c) -> e h c", h=2); o += OC
        ow_t = ep_t[:, o:o + L * OC].rearrange(
            "e (h l c) -> e h l c", h=2, l=L); o += L * OC
        ob_t = ep_t[:, o:o + L]; o += L
        gtz_t = ep_t[:, o:o + P]                 # gates.T | zeros

        # ---- small matmuls first so their consumers unblock early ----
        # sel_db^h [64h+b, hc] ; sel_ow^h [64h+b, (l, hc)] ; sel_ob [p, l]
        psum_db = pss.tile([P, HC], F32)
        psum_ow = pss.tile([P, L, HC], F32)
        for h in range(2):
            sl = slice(h * 64, h * 64 + 64)
            nc.tensor.matmul(psum_db[sl, :], gt_t, db_t[:, h, :],
                             start=True, stop=True, skip_group_check=True)
            nc.tensor.matmul(
                psum_ow[sl, :, :].rearrange("b l c -> b (l c)"),
                gt_t, ow_t[:, h].rearrange("e l c -> e (l c)"),
                start=True, stop=True, skip_group_check=True,
            )
        psum_ob = pss.tile([P, L], F32)
        nc.tensor.matmul(psum_ob[:], gtz_t, ob_t, start=True, stop=True)
        sdb_t = smallp.tile([P, HC], F32)
        nc.scalar.copy(sdb_t[:], psum_db[:])
        sob_t = smallp.tile([P, L], F32)
        nc.scalar.copy(sob_t[:], psum_ob[:])

        # ---- stage 1: y[64h+b, (e, hc)] = x . dense_w[e, oc_half, :] ----
        # The h=0 / h=1 matmuls write PSUM partition halves 0-63 / 64-127,
        # i.e. disjoint PE col-groups -> the PE overlaps the two fp32
        # streams (2x throughput).  k-outer so the PE consumes each w1
        # chunk as it lands.
        # Gate-broadcast table gb[p, (e, hc)] = g[b, e], built early on the
        # DVE (hidden under the w1 DMA stream).
        ones_t = smallp.tile([P, HC], F32)
        nc.vector.memset(ones_t[:], 1.0)
        gb_t = consts.tile([P, E, HC], F32)
        for e in range(E):
            nc.vector.tensor_scalar_mul(gb_t[:, e, :], ones_t[:],
                                        gc_t[:, e:e + 1])

        psum_y = psy.tile([P, E, HC], F32)
        for k in range(KT):
            for h in range(2):
                nc.tensor.matmul(
                    psum_y[h * 64:h * 64 + 64, :, :].rearrange(
                        "b e c -> b (e c)"),
                    xt_t[:, k, :],
                    w1_t[:, k, h].rearrange("p e c -> p (e c)"),
                    start=(k == 0),
                    stop=(k == KT - 1),
                    skip_group_check=True,
                )

        prod_t = mixp.tile([P, E, HC], F32)
        nc.vector.tensor_tensor(
            out=prod_t[:], in0=psum_y[:], in1=gb_t[:], op=OP.mult,
        )
        # contiguous pairwise tree over e (strided reduce is ~2x slower)
        t1 = mixp.tile([P, 4, HC], F32)
        nc.vector.tensor_add(t1[:], prod_t[:, 0:4, :], prod_t[:, 4:8, :])
        t2 = mixp.tile([P, 2, HC], F32)
        nc.vector.tensor_add(t2[:], t1[:, 0:2, :], t1[:, 2:4, :])
        t3 = mixp.tile([P, HC], F32)
        nc.vector.tensor_add(t3[:], t2[:, 0, :], t2[:, 1, :])
        acc = mixp.tile([P, HC], F32)
        nc.vector.tensor_add(acc[:], t3[:], sdb_t[:])

        t_t = smallp.tile([P, HC], F32)
        nc.scalar.activation(t_t[:], acc[:], AF.Tanh)

        # ---- stage 2: partial[64h+b, l] = sum_hc t * sel_ow (+ sel_ob) ----
        # NOTE: InstTensorTensorReduce faults TRN2; scalar_tensor_tensor with
        # accum_out (free-dim sum) is the reliable path.
        out_t = smallp.tile([P, L], F32)
        pre_t = smallp.tile([P, L], F32)
        dump = smallp.tile([P, HC], F32)
        for l in range(L):
            nc.vector.scalar_tensor_tensor(
                out=dump[:],
                in0=psum_ow[:, l, :],
                scalar=1.0,
                in1=t_t[:],
                op0=OP.mult,
                op1=OP.mult,
                accum_out=pre_t[:, l:l + 1],
            )
        nc.vector.tensor_add(out_t[:], pre_t[:], sob_t[:])

        nc.sync.dma_start(out=out_d.ap(), in_=out_t[:])

    nc.compile()
    return nc


def _prep_inputs(X, gates, dense_w, dense_b, out_w, out_b):
    """Host-side layout prep (slice/transpose/cast) -> per-core input maps."""
    import ml_dtypes

    BF = ml_dtypes.bfloat16
    X = np.asarray(X, dtype=np.float32)
    gates = np.asarray(gates, dtype=np.float32)
    dense_w = np.asarray(dense_w, dtype=np.float32)
    dense_b = np.asarray(dense_b, dtype=np.float32)
    out_w = np.asarray(out_w, dtype=np.float32)
    out_b = np.asarray(out_b, dtype=np.float32)

    xcls = X[:, 0, :]                                     # [B, H]
    # xt[i_lo, k, b] = x[b, k*128 + i_lo]
    xt = np.ascontiguousarray(
        xcls.T.reshape(KT, P, B).transpose(1, 0, 2).astype(BF))
    gt = np.ascontiguousarray(gates.T)                    # [E, B]
    gtz = np.concatenate([gt, np.zeros_like(gt)], axis=1)  # [E, 128]
    gc2 = np.ascontiguousarray(np.vstack([gates, gates]))  # [128, E]

    in_maps = []
    for c in range(NCORES):
        sl = slice(c * OC, (c + 1) * OC)
        # w1[i_lo, k, h, e, hc] = dense_w[e, c*OC + h*64 + hc, k*128 + i_lo]
        w1 = np.ascontiguousarray(
            dense_w[:, sl, :]                   # [E, OC, H]
            .reshape(E, 2, HC, KT, P)           # [e, h, hc, k, i_lo]
            .transpose(4, 3, 1, 0, 2)           # [i_lo, k, h, e, hc]
            .astype(BF)
        )

        # ow2[e, (h, l, hc)] = out_w[e, l, c*OC + h*64 + hc]
        ow2 = (out_w[:, :, sl].reshape(E, L, 2, HC)
               .transpose(0, 2, 1, 3).reshape(E, L * OC))
        ob = out_b if c == 0 else np.zeros_like(out_b)
        ep = np.ascontiguousarray(
            np.concatenate([gt, dense_b[:, sl], ow2, ob, gtz], axis=1)
        )
        in_maps.append({
            "xt": xt,
            "w1": w1,
            "ep": ep,
            "gc": gc2,
        })
    return in_maps


def _run(in_maps, trace=False, tmpdir=None):
    global _cached
    from concourse.bass_utils import run_bass_kernel_spmd

    if _cached is None:
        _cached = _build()
    res = run_bass_kernel_spmd(
        _cached, in_maps, list(range(NCORES)), trace=trace, tmpdir=tmpdir,
    )
    return res


def kernel(X, gates, dense_w, dense_b, out_w, out_b):
    in_maps = _prep_inputs(X, gates, dense_w, dense_b, out_w, out_b)
    res = _run(in_maps)
    acc = np.zeros((B, L), dtype=np.float64)
    for c in range(NCORES):
        part = res.results[c]["out"].astype(np.float64)   # [128, L]
        acc += part.reshape(2, B, L).sum(axis=0)
    return acc.astype(np.float32)



# revision 8
# speedup vs baseline: 1.3000x; 1.0448x over previous
"""Trainium2 Bass kernel for nn_MoEsparseRoutingForClassification.

Reference computation (B=64, S=128, H=1024, E=8, L=2):
    x = X[:, 0, :]                                   # CLS token [B,H]
    y[b,o]   = sum_e g[b,e] * (x[b] . dense_w[e,o,:]) + (g @ dense_b)[b,o]
    t        = tanh(y)
    out[b,l] = sum_e g[b,e] * (t[b] . out_w[e,l,:])  + (g @ out_b)[b,l]

Distribution: the H output dim of the dense layer is sharded 8 ways
(OC=128 per core).  Core c computes y[:, c*OC:(c+1)*OC] (which needs the
full CLS token but only a slice dense_w[:, c_slice, :]), applies tanh,
and contracts its slice against out_w[:, :, c_slice] to produce a
partial [L, 128] logit tile.  The partials (incl. the out_b bias, fed
only to core 0) sum to the full output on the host.  No cross-core
collective is needed.

v3 notes (trace-driven):
- w1/xt ship as bf16 (host cast): halves the dominant HBM train and makes
  stage-1 matmuls single-pass.  Measured rel err vs fp32 ref: ~2.3e-3
  (tolerance 2e-2).
- gc (gate columns + a 128x128 fp32 identity) goes FIRST on the sync ring:
  small transfers issued after the w1 flood starve at ~1.5% BW (packet
  round-robin), which starved the gate-broadcast build in v2.
- ~18 dummy warm-up matmuls run during the DMA train so HAM un-throttles
  the PE (cold MMs run at 1.2 GHz, 2x slower).  They write psum_y with
  start&stop so the real k=0 (start=True) ordering is enforced via WAW.
- Output is produced TRANSPOSED as [L, 128] on 2 partitions: the [128, 2]
  layout needed 128 8-byte HBM-write descriptors whose completion receipt
  cost ~2.5us; 2 partitions x 512B needs 2.  The transpose rides the PE
  (pre_t.T @ I) and accumulates straight onto the sel_ob partial in PSUM,
  fusing the bias add.
- enable_partition_id=False drops the per-engine partition-id loads from
  the BSP preamble.
"""

import sys

import numpy as np

for _p in ("/opt/trn_rl_repo",):
    if _p not in sys.path:
        sys.path.insert(0, _p)

# If the environment sets BASS_TRACE but lacks antenv.axon_hooks (this agent
# image does), run_bass_kernel_spmd would crash on import; pre-seed a no-op
# module so tracing degrades gracefully instead.
try:  # pragma: no cover
    import antenv.axon_hooks  # noqa: F401
except Exception:  # pragma: no cover
    import types as _types

    _m = _types.ModuleType("antenv.axon_hooks")
    _m._hook = None
    _m.set_axon_ntff_profile_hook = lambda h: setattr(_m, "_hook", h)
    _m.get_axon_ntff_profile_hook = lambda: _m._hook
    sys.modules["antenv.axon_hooks"] = _m

B, S, H = 64, 128, 1024
E, L = 8, 2
NCORES = 8
OC = H // NCORES          # dense-output slice per core (128)
HC = OC // 2              # half-slice mapped to a PSUM partition half (64)
KT = H // 128             # contraction tiles
P = 128
W1_CHUNKS = ((0, 2), (2, 5), (5, 7), (7, 8))   # small last chunk -> short trail
N_WARMUP = 18             # dummy MMs (N=256) to hold the PE HAM un-throttled

_cached = None


def _build():
    from contextlib import ExitStack

    import concourse.tile as tile
    from concourse import bacc, mybir

    F32 = mybir.dt.float32
    BF16 = mybir.dt.bfloat16
    AF = mybir.ActivationFunctionType
    OP = mybir.AluOpType

    nc = bacc.Bacc("TRN2", target_bir_lowering=False, debug=False,
                   num_devices=NCORES, enable_partition_id=False)

    # E-pack along the free dim (one DMA for all E-partition consts):
    #   gt [E,B] | db [E,2,HC] | ow2 [E,2,L,HC] | ob [E,L] | gtz [E,P]
    EPACK = B + OC + L * OC + L + P      # 64+128+256+2+128 = 578
    xt_d = nc.dram_tensor("xt", [P, KT, B], BF16, kind="ExternalInput")
    w1_d = nc.dram_tensor("w1", [P, KT, 2, E, HC], BF16, kind="ExternalInput")
    ep_d = nc.dram_tensor("ep", [E, EPACK], F32, kind="ExternalInput")
    gc_d = nc.dram_tensor("gc", [P, E + P], F32, kind="ExternalInput")
    out_d = nc.dram_tensor("out", [L, P], F32, kind="ExternalOutput")

    with tile.TileContext(nc) as tc, ExitStack() as ctx:
        consts = ctx.enter_context(tc.tile_pool(name="consts", bufs=1))
        wpool = ctx.enter_context(tc.tile_pool(name="wpool", bufs=1))
        mixp = ctx.enter_context(tc.tile_pool(name="mixp", bufs=1))
        smallp = ctx.enter_context(tc.tile_pool(name="smallp", bufs=1))
        psy = ctx.enter_context(tc.tile_pool(name="psy", bufs=1, space="PSUM"))
        pss = ctx.enter_context(tc.tile_pool(name="pss", bufs=1, space="PSUM"))

        # Sync ring order: gc (tiny, feeds the gate-broadcast build early),
        # xt, then the w1 train.  ep rides the scalar ring concurrently.
        gc_t = consts.tile([P, E + P], F32)
        nc.sync.dma_start(out=gc_t, in_=gc_d.ap())
        ident_t = gc_t[:, E:]                    # [128,128] fp32 identity
        xt_t = consts.tile([P, KT, B], BF16)
        nc.sync.dma_start(out=xt_t, in_=xt_d.ap())
        w1_t = wpool.tile([P, KT, 2, E, HC], BF16)
        for klo, khi in W1_CHUNKS:
            nc.sync.dma_start(
                out=w1_t[:, klo:khi],
                in_=w1_d.ap()[:, klo:khi],
            )
        ep_t = consts.tile([E, EPACK], F32)
        nc.scalar.dma_start(out=ep_t, in_=ep_d.ap())
        o = 0
        gt_t = ep_t[:, o:o + B]; o += B
        db_t = ep_t[:, o:o + OC].rearrange("e (h c) -> e h c", h=2); o += OC
        ow_t = ep_t[:, o:o + L * OC].rearrange(
            "e (h l c) -> e h l c", h=2, l=L); o += L * OC
        ob_t = ep_t[:, o:o + L]; o += L
        gtz_t = ep_t[:, o:o + P]                 # gates.T | zeros

        # ---- PE warm-up: dummy MMs fill the DMA-train window so HAM
        # un-throttles (1.2 -> 2.4 GHz) before the real stream.  WAW on
        # psum_y orders them before the real k-loop.
        dummy_t = smallp.tile([P, 256], BF16)
        nc.vector.memset(dummy_t[:], 0.0)
        psum_y = psy.tile([P, E, HC], F32)
        dummy_ps = psum_y.rearrange("b e c -> b (e c)")[:, 0:256]
        for _ in range(N_WARMUP):
            nc.tensor.matmul(dummy_ps, dummy_t[:, 0:128], dummy_t[:],
                             start=True, stop=True, skip_group_check=True)

        # ---- small matmuls next so their consumers unblock early ----
        # sel_db^h [64h+b, hc] ; sel_ow^h [64h+b, (l, hc)] ; sel_obT [l, p]
        psum_db = pss.tile([P, HC], F32)
        psum_ow = pss.tile([P, L, HC], F32)
        for h in range(2):
            sl = slice(h * 64, h * 64 + 64)
            nc.tensor.matmul(psum_db[sl, :], gt_t, db_t[:, h, :],
                             start=True, stop=True, skip_group_check=True)
            nc.tensor.matmul(
                psum_ow[sl, :, :].rearrange("b l c -> b (l c)"),
                gt_t, ow_t[:, h].rearrange("e l c -> e (l c)"),
                start=True, stop=True, skip_group_check=True,
            )
        # transposed sel_ob partial: [l, 64h+b] (gtz zeroes the h=1 copy so
        # the host h-sum counts ob once); pre.T accumulates onto it later.
        psum_oT = pss.tile([L, P], F32)
        nc.tensor.matmul(psum_oT[:], ob_t, gtz_t,
                         start=True, stop=False, skip_group_check=True)
        sdb_t = smallp.tile([P, HC], F32)
        nc.scalar.copy(sdb_t[:], psum_db[:])

        # ---- stage 1: y[64h+b, (e, hc)] = x . dense_w[e, oc_half, :] ----
        # The h=0 / h=1 matmuls write PSUM partition halves 0-63 / 64-127,
        # i.e. disjoint PE col-groups -> the two bf16 streams overlap.
        # k-outer so the PE consumes each w1 chunk as it lands.
        # Gate-broadcast table gb[p, (e, hc)] = g[b, e], built early on the
        # DVE (hidden under the w1 DMA stream).
        ones_t = smallp.tile([P, HC], F32)
        nc.vector.memset(ones_t[:], 1.0)
        gb_t = consts.tile([P, E, HC], F32)
        for e in range(E):
            nc.vector.tensor_scalar_mul(gb_t[:, e, :], ones_t[:],
                                        gc_t[:, e:e + 1])

        for k in range(KT):
            for h in range(2):
                nc.tensor.matmul(
                    psum_y[h * 64:h * 64 + 64, :, :].rearrange(
                        "b e c -> b (e c)"),
                    xt_t[:, k, :],
                    w1_t[:, k, h].rearrange("p e c -> p (e c)"),
                    start=(k == 0),
                    stop=(k == KT - 1),
                    skip_group_check=True,
                )

        # gate-mix: prod in bf16 so the pairwise tree runs in 2x DVE mode
        prod_t = mixp.tile([P, E, HC], BF16)
        nc.vector.tensor_tensor(
            out=prod_t[:], in0=psum_y[:], in1=gb_t[:], op=OP.mult,
        )
        t1 = mixp.tile([P, 4, HC], BF16)
        nc.vector.tensor_add(t1[:], prod_t[:, 0:4, :], prod_t[:, 4:8, :])
        t2 = mixp.tile([P, 2, HC], BF16)
        nc.vector.tensor_add(t2[:], t1[:, 0:2, :], t1[:, 2:4, :])
        t3 = mixp.tile([P, HC], BF16)
        nc.vector.tensor_add(t3[:], t2[:, 0, :], t2[:, 1, :])
        acc = mixp.tile([P, HC], F32)
        nc.vector.tensor_add(acc[:], t3[:], sdb_t[:])

        t_t = smallp.tile([P, HC], F32)
        nc.scalar.activation(t_t[:], acc[:], AF.Tanh)

        # ---- stage 2: pre[64h+b, l] = sum_hc t * sel_ow ----
        # NOTE: InstTensorTensorReduce faults TRN2; scalar_tensor_tensor with
        # accum_out (free-dim sum) is the reliable path.
        pre_t = smallp.tile([P, L], F32)
        dump = smallp.tile([P, HC], F32)
        for l in range(L):
            nc.vector.scalar_tensor_tensor(
                out=dump[:],
                in0=psum_ow[:, l, :],
                scalar=1.0,
                in1=t_t[:],
                op0=OP.mult,
                op1=OP.mult,
                accum_out=pre_t[:, l:l + 1],
            )
        # transpose pre onto the sel_ob partial: psum_oT += pre.T
        # (plain matmul with identity rhs; PSUM accumulation does the add)
        nc.tensor.matmul(psum_oT[:], pre_t[:], ident_t,
                         start=False, stop=True, skip_group_check=True)
        outT_t = smallp.tile([L, P], F32)
        nc.scalar.copy(outT_t[:], psum_oT[:])

        nc.sync.dma_start(out=out_d.ap(), in_=outT_t[:])

    nc.compile()
    return nc


def _prep_inputs(X, gates, dense_w, dense_b, out_w, out_b):
    """Host-side layout prep (slice/transpose/cast) -> per-core input maps."""
    import ml_dtypes

    BF = ml_dtypes.bfloat16
    X = np.asarray(X, dtype=np.float32)
    gates = np.asarray(gates, dtype=np.float32)
    dense_w = np.asarray(dense_w, dtype=np.float32)
    dense_b = np.asarray(dense_b, dtype=np.float32)
    out_w = np.asarray(out_w, dtype=np.float32)
    out_b = np.asarray(out_b, dtype=np.float32)

    xcls = X[:, 0, :]                                     # [B, H]
    # xt[i_lo, k, b] = x[b, k*128 + i_lo]
    xt = np.ascontiguousarray(
        xcls.T.reshape(KT, P, B).transpose(1, 0, 2).astype(BF))
    gt = np.ascontiguousarray(gates.T)                    # [E, B]
    gtz = np.concatenate([gt, np.zeros_like(gt)], axis=1)  # [E, 128]
    # gc: gate columns (dup across the two PSUM halves) | fp32 identity
    gc2 = np.ascontiguousarray(np.concatenate(
        [np.vstack([gates, gates]), np.eye(P, dtype=np.float32)], axis=1))

    in_maps = []
    for c in range(NCORES):
        sl = slice(c * OC, (c + 1) * OC)
        # w1[i_lo, k, h, e, hc] = dense_w[e, c*OC + h*64 + hc, k*128 + i_lo]
        w1 = np.ascontiguousarray(
            dense_w[:, sl, :]                   # [E, OC, H]
            .reshape(E, 2, HC, KT, P)           # [e, h, hc, k, i_lo]
            .transpose(4, 3, 1, 0, 2)           # [i_lo, k, h, e, hc]
            .astype(BF)
        )

        # ow2[e, (h, l, hc)] = out_w[e, l, c*OC + h*64 + hc]
        ow2 = (out_w[:, :, sl].reshape(E, L, 2, HC)
               .transpose(0, 2, 1, 3).reshape(E, L * OC))
        ob = out_b if c == 0 else np.zeros_like(out_b)
        ep = np.ascontiguousarray(
            np.concatenate([gt, dense_b[:, sl], ow2, ob, gtz], axis=1)
        )
        in_maps.append({
            "xt": xt,
            "w1": w1,
            "ep": ep,
            "gc": gc2,
        })
    return in_maps


def _run(in_maps, trace=False, tmpdir=None):
    global _cached
    from concourse.bass_utils import run_bass_kernel_spmd

    if _cached is None:
        _cached = _build()
    res = run_bass_kernel_spmd(
        _cached, in_maps, list(range(NCORES)), trace=trace, tmpdir=tmpdir,
    )
    return res


def kernel(X, gates, dense_w, dense_b, out_w, out_b):
    in_maps = _prep_inputs(X, gates, dense_w, dense_b, out_w, out_b)
    res = _run(in_maps)
    acc = np.zeros((B, L), dtype=np.float64)
    for c in range(NCORES):
        part = res.results[c]["out"].astype(np.float64)   # [L, 128]
        acc += part.reshape(L, 2, B).sum(axis=1).T
    return acc.astype(np.float32)


# revision 17
# speedup vs baseline: 1.3048x; 1.0037x over previous
"""Trainium2 Bass kernel for nn_MoEsparseRoutingForClassification.

Reference computation (B=64, S=128, H=1024, E=8, L=2):
    x = X[:, 0, :]                                   # CLS token [B,H]
    y[b,o]   = sum_e g[b,e] * (x[b] . dense_w[e,o,:]) + (g @ dense_b)[b,o]
    t        = tanh(y)
    out[b,l] = sum_e g[b,e] * (t[b] . out_w[e,l,:])  + (g @ out_b)[b,l]

Distribution: the H output dim of the dense layer is sharded 8 ways
(OC=128 per core).  Core c computes y[:, c*OC:(c+1)*OC] (which needs the
full CLS token but only a slice dense_w[:, c_slice, :]), applies tanh,
and contracts its slice against out_w[:, :, c_slice] to produce a
partial [L, 128] logit tile.  The partials (incl. the out_b bias, fed
only to core 0) sum to the full output on the host.  No cross-core
collective is needed.

v3 notes (trace-driven):
- w1/xt ship as bf16 (host cast): halves the dominant HBM train and makes
  stage-1 matmuls single-pass.  Measured rel err vs fp32 ref: ~2.3e-3
  (tolerance 2e-2).
- gc (gate columns + a 128x128 fp32 identity) goes FIRST on the sync ring:
  small transfers issued after the w1 flood starve at ~1.5% BW (packet
  round-robin), which starved the gate-broadcast build in v2.
- ~18 dummy warm-up matmuls run during the DMA train so HAM un-throttles
  the PE (cold MMs run at 1.2 GHz, 2x slower).  They write psum_y with
  start&stop so the real k=0 (start=True) ordering is enforced via WAW.
- Output is produced TRANSPOSED as [L, 128] on 2 partitions: the [128, 2]
  layout needed 128 8-byte HBM-write descriptors whose completion receipt
  cost ~2.5us; 2 partitions x 512B needs 2.  The transpose rides the PE
  (pre_t.T @ I) and accumulates straight onto the sel_ob partial in PSUM,
  fusing the bias add.
- enable_partition_id=False drops the per-engine partition-id loads from
  the BSP preamble.
"""

import sys

import numpy as np

for _p in ("/opt/trn_rl_repo",):
    if _p not in sys.path:
        sys.path.insert(0, _p)

# If the environment sets BASS_TRACE but lacks antenv.axon_hooks (this agent
# image does), run_bass_kernel_spmd would crash on import; pre-seed a no-op
# module so tracing degrades gracefully instead.
try:  # pragma: no cover
    import antenv.axon_hooks  # noqa: F401
except Exception:  # pragma: no cover
    import types as _types

    _m = _types.ModuleType("antenv.axon_hooks")
    _m._hook = None
    _m.set_axon_ntff_profile_hook = lambda h: setattr(_m, "_hook", h)
    _m.get_axon_ntff_profile_hook = lambda: _m._hook
    sys.modules["antenv.axon_hooks"] = _m

B, S, H = 64, 128, 1024
E, L = 8, 2
NCORES = 8
OC = H // NCORES          # dense-output slice per core (128)
HC = OC // 2              # half-slice mapped to a PSUM partition half (64)
KT = H // 128             # contraction tiles
P = 128
XT_ELEMS = KT * B         # 512 bf16 elems/partition, rides at the train head
KP = 2 * E * HC           # elems per k-plane per partition (1024)
# w1 k-plane groups per DMA chunk; xt is fused into chunk 0 so it moves in
# large packets instead of a slow 1KB-packet prefix.
W1_CHUNKS = ((0, 3), (3, 6), (6, 8))
# zero-matmuls (rhs=0, accumulate) inserted after each chunk's k-group to
# keep the PE busy while waiting for the next chunk sem: idle >~1.2us was
# observed to re-throttle HAM back to 1.2 GHz.
N_ZFILL = (3, 3)

_cached = None


def _build():
    from contextlib import ExitStack

    import concourse.tile as tile
    from concourse import bacc, mybir

    F32 = mybir.dt.float32
    BF16 = mybir.dt.bfloat16
    AF = mybir.ActivationFunctionType
    OP = mybir.AluOpType

    nc = bacc.Bacc("TRN2", target_bir_lowering=False, debug=False,
                   num_devices=NCORES, enable_partition_id=False)

    # E-pack along the free dim (one DMA for all E-partition consts):
    #   gt [E,B] | db [E,2,HC] | ow2 [E,2,L,HC] | ob [E,L] | gtz [E,P]
    EPACK = B + OC + L * OC + L + P      # 64+128+256+2+128 = 578
    wx_d = nc.dram_tensor("wx", [P, XT_ELEMS + KT * KP], BF16,
                          kind="ExternalInput")
    ep_d = nc.dram_tensor("ep", [E, EPACK], F32, kind="ExternalInput")
    gc_d = nc.dram_tensor("gc", [P, E + P], F32, kind="ExternalInput")
    out_d = nc.dram_tensor("out", [L, P], F32, kind="ExternalOutput")

    with tile.TileContext(nc) as tc, ExitStack() as ctx:
        consts = ctx.enter_context(tc.tile_pool(name="consts", bufs=1))
        wpool = ctx.enter_context(tc.tile_pool(name="wpool", bufs=1))
        mixp = ctx.enter_context(tc.tile_pool(name="mixp", bufs=1))
        smallp = ctx.enter_context(tc.tile_pool(name="smallp", bufs=1))
        psy = ctx.enter_context(tc.tile_pool(name="psy", bufs=1, space="PSUM"))
        pss = ctx.enter_context(tc.tile_pool(name="pss", bufs=1, space="PSUM"))

        # Sync ring: the wx train (xt fused at the head of chunk 0, then w1
        # k-planes), then gc at the tail (its gates/identity are consumed
        # only after the train).  ep rides the scalar ring concurrently.
        wx_t = wpool.tile([P, XT_ELEMS + KT * KP], BF16)
        xt_t = wx_t[:, 0:XT_ELEMS].rearrange("p (k b) -> p k b", k=KT)
        w1_t = wx_t[:, XT_ELEMS:].rearrange(
            "p (k h e c) -> p k h e c", k=KT, h=2, e=E)
        bounds = [0] + [XT_ELEMS + khi * KP for _, khi in W1_CHUNKS]
        for lo, hi in zip(bounds[:-1], bounds[1:]):
            nc.sync.dma_start(out=wx_t[:, lo:hi], in_=wx_d.ap()[:, lo:hi])
        gc_t = consts.tile([P, E + P], F32)
        nc.sync.dma_start(out=gc_t, in_=gc_d.ap())
        ident_t = gc_t[:, E:]                    # [128,128] fp32 identity
        ep_t = consts.tile([E, EPACK], F32)
        nc.scalar.dma_start(out=ep_t, in_=ep_d.ap())
        o = 0
        gt_t = ep_t[:, o:o + B]; o += B
        db_t = ep_t[:, o:o + OC].rearrange("e (h c) -> e h c", h=2); o += OC
        ow_t = ep_t[:, o:o + L * OC].rearrange(
            "e (h l c) -> e h l c", h=2, l=L); o += L * OC
        ob_t = ep_t[:, o:o + L]; o += L
        gtz_t = ep_t[:, o:o + P]                 # gates.T | zeros

        zero_t = smallp.tile([P, 256], BF16)
        nc.vector.memset(zero_t[:], 0.0)
        psum_y = psy.tile([P, E, HC], F32)

        # ---- small matmuls first so their consumers unblock early ----
        # sel_db^h [64h+b, hc] ; sel_ow^h [64h+b, (l, hc)] ; sel_obT [l, p]
        psum_db = pss.tile([P, HC], F32)
        psum_ow = pss.tile([P, L, HC], F32)
        for h in range(2):
            sl = slice(h * 64, h * 64 + 64)
            nc.tensor.matmul(psum_db[sl, :], gt_t, db_t[:, h, :],
                             start=True, stop=True, skip_group_check=True)
            nc.tensor.matmul(
                psum_ow[sl, :, :].rearrange("b l c -> b (l c)"),
                gt_t, ow_t[:, h].rearrange("e l c -> e (l c)"),
                start=True, stop=True, skip_group_check=True,
            )
        # transposed sel_ob partial: [l, 64h+b] (gtz zeroes the h=1 copy so
        # the host h-sum counts ob once); pre.T accumulates onto it later.
        psum_oT = pss.tile([L, P], F32)
        nc.tensor.matmul(psum_oT[:], ob_t, gtz_t,
                         start=True, stop=False, skip_group_check=True)
        sdb_t = smallp.tile([P, HC], F32)
        nc.scalar.copy(sdb_t[:], psum_db[:])

        # ---- stage 1: y[64h+b, (e, hc)] = x . dense_w[e, oc_half, :] ----
        # The h=0 / h=1 matmuls write PSUM partition halves 0-63 / 64-127,
        # i.e. disjoint PE col-groups -> the two bf16 streams overlap.
        # k-outer so the PE consumes each w1 chunk as it lands.
        # Gate-broadcast table gb[p, (e, hc)] = g[b, e], built early on the
        # DVE (hidden under the w1 DMA stream).
        ones_t = smallp.tile([P, HC], F32)
        nc.vector.memset(ones_t[:], 1.0)
        gb_t = consts.tile([P, E, HC], F32)
        for e in range(E):
            nc.vector.tensor_scalar_mul(gb_t[:, e, :], ones_t[:],
                                        gc_t[:, e:e + 1])

        psum_y0 = psum_y.rearrange("b e c -> b (e c)")
        for ci, (klo, khi) in enumerate(W1_CHUNKS):
            for k in range(klo, khi):
                for h in range(2):
                    nc.tensor.matmul(
                        psum_y[h * 64:h * 64 + 64, :, :].rearrange(
                            "b e c -> b (e c)"),
                        xt_t[:, k, :],
                        w1_t[:, k, h].rearrange("p e c -> p (e c)"),
                        start=(k == 0),
                        stop=(k == KT - 1),
                        skip_group_check=True,
                    )
            if ci < len(N_ZFILL):
                # accumulate-zero matmuls: keep the PE HAM-warm across the
                # wait for the next chunk sem (same lhsT -> no reload).
                for _ in range(N_ZFILL[ci]):
                    nc.tensor.matmul(
                        psum_y0[0:64, 0:256], xt_t[:, khi - 1, :], zero_t[:],
                        start=False, stop=False, skip_group_check=True,
                    )

        # gate-mix: prod in bf16 so the pairwise tree runs in 2x DVE mode
        prod_t = mixp.tile([P, E, HC], BF16)
        nc.vector.tensor_tensor(
            out=prod_t[:], in0=psum_y[:], in1=gb_t[:], op=OP.mult,
        )
        t1 = mixp.tile([P, 4, HC], BF16)
        nc.vector.tensor_add(t1[:], prod_t[:, 0:4, :], prod_t[:, 4:8, :])
        t2 = mixp.tile([P, 2, HC], BF16)
        nc.vector.tensor_add(t2[:], t1[:, 0:2, :], t1[:, 2:4, :])
        t3 = mixp.tile([P, HC], BF16)
        nc.vector.tensor_add(t3[:], t2[:, 0, :], t2[:, 1, :])
        acc = mixp.tile([P, HC], F32)
        nc.vector.tensor_add(acc[:], t3[:], sdb_t[:])

        t_t = smallp.tile([P, HC], F32)
        nc.scalar.activation(t_t[:], acc[:], AF.Tanh)

        # ---- stage 2: pre[64h+b, l] = sum_hc t * sel_ow ----
        # NOTE: InstTensorTensorReduce faults TRN2; scalar_tensor_tensor with
        # accum_out (free-dim sum) is the reliable path.
        pre_t = smallp.tile([P, L], F32)
        dump = smallp.tile([P, HC], F32)
        for l in range(L):
            nc.vector.scalar_tensor_tensor(
                out=dump[:],
                in0=psum_ow[:, l, :],
                scalar=1.0,
                in1=t_t[:],
                op0=OP.mult,
                op1=OP.mult,
                accum_out=pre_t[:, l:l + 1],
            )
        # transpose pre onto the sel_ob partial: psum_oT += pre.T
        # (PE transpose datapath; PSUM accumulation does the add)
        nc.tensor.matmul(psum_oT[:], pre_t[:], ident_t, is_transpose=True,
                         start=False, stop=True, skip_group_check=True)
        outT_t = smallp.tile([L, P], F32)
        nc.scalar.copy(outT_t[:], psum_oT[:])

        nc.sync.dma_start(out=out_d.ap(), in_=outT_t[:])

    nc.compile()
    return nc


def _prep_inputs(X, gates, dense_w, dense_b, out_w, out_b):
    """Host-side layout prep (slice/transpose/cast) -> per-core input maps."""
    import ml_dtypes

    BF = ml_dtypes.bfloat16
    X = np.asarray(X, dtype=np.float32)
    gates = np.asarray(gates, dtype=np.float32)
    dense_w = np.asarray(dense_w, dtype=np.float32)
    dense_b = np.asarray(dense_b, dtype=np.float32)
    out_w = np.asarray(out_w, dtype=np.float32)
    out_b = np.asarray(out_b, dtype=np.float32)

    xcls = X[:, 0, :]                                     # [B, H]
    # xt[i_lo, k, b] = x[b, k*128 + i_lo]
    xt = (xcls.T.reshape(KT, P, B).transpose(1, 0, 2)
          .astype(BF).reshape(P, XT_ELEMS))
    gt = np.ascontiguousarray(gates.T)                    # [E, B]
    gtz = np.concatenate([gt, np.zeros_like(gt)], axis=1)  # [E, 128]
    # gc: gate columns (dup across the two PSUM halves) | fp32 identity
    gc2 = np.ascontiguousarray(np.concatenate(
        [np.vstack([gates, gates]), np.eye(P, dtype=np.float32)], axis=1))

    in_maps = []
    for c in range(NCORES):
        sl = slice(c * OC, (c + 1) * OC)
        # w1[i_lo, k, h, e, hc] = dense_w[e, c*OC + h*64 + hc, k*128 + i_lo]
        w1 = (dense_w[:, sl, :]                 # [E, OC, H]
              .reshape(E, 2, HC, KT, P)         # [e, h, hc, k, i_lo]
              .transpose(4, 3, 1, 0, 2)         # [i_lo, k, h, e, hc]
              .astype(BF).reshape(P, KT * KP))
        wx = np.ascontiguousarray(np.concatenate([xt, w1], axis=1))

        # ow2[e, (h, l, hc)] = out_w[e, l, c*OC + h*64 + hc]
        ow2 = (out_w[:, :, sl].reshape(E, L, 2, HC)
               .transpose(0, 2, 1, 3).reshape(E, L * OC))
        ob = out_b if c == 0 else np.zeros_like(out_b)
        ep = np.ascontiguousarray(
            np.concatenate([gt, dense_b[:, sl], ow2, ob, gtz], axis=1)
        )
        in_maps.append({
            "wx": wx,
            "ep": ep,
            "gc": gc2,
        })
    return in_maps


def _run(in_maps, trace=False, tmpdir=None):
    global _cached
    from concourse.bass_utils import run_bass_kernel_spmd

    if _cached is None:
        _cached = _build()
    res = run_bass_kernel_spmd(
        _cached, in_maps, list(range(NCORES)), trace=trace, tmpdir=tmpdir,
    )
    return res


def kernel(X, gates, dense_w, dense_b, out_w, out_b):
    in_maps = _prep_inputs(X, gates, dense_w, dense_b, out_w, out_b)
    res = _run(in_maps)
    acc = np.zeros((B, L), dtype=np.float64)
    for c in range(NCORES):
        part = res.results[c]["out"].astype(np.float64)   # [L, 128]
        acc += part.reshape(L, 2, B).sum(axis=1).T
    return acc.astype(np.float32)
